# revision 1
# baseline (speedup 1.0000x reference)
"""GCN message-passing kernel for 8 Trainium2 NeuronCores (Bass/Tile).

v1 redesign vs baseline:
- 4 SWDGE queues round-robin for gathers (4x ring-drain parallelism; the
  per-queue DMA ring drains at ~7.5ns/descriptor and was the bottleneck).
- One gather per (subwindow, stream): TT tiles = TT*128 rows, sized to stay
  under the ring capacity so preps never head-of-line block the Q7 engine.
- h table in fp8e4 (halves HBM + AllGather bytes); layer-0 tables scaled x64
  to clear the fp8 subnormal range (BN absorbs any per-layer input scale).
- Split runs across tiles (a node's edges may span tiles; PSUM accumulates),
  so tiles per (core,sw,stream) = ceil(edges/128) and padding is minimal.
- Node bins balanced on BOTH (L,H) stream degree simultaneously.
- Update/stats matmuls in bf16 (4x faster than f32 on PE).
- countT/deginv/t512/W resident in SBUF across layers; hlin kept in SBUF.
- AllGather output tensor Shared; bond-encoder matmuls issued first per
  window so PE has work while the AllGather completes.
"""
import sys

sys.path.insert(0, "/opt/trn_rl_repo")

import os

import numpy as np
import ml_dtypes

import concourse.bass as bass
import concourse.bacc as bacc
import concourse.mybir as mybir
import concourse.tile as tile
from concourse.bass_utils import run_bass_kernel_spmd

P = 128
WSZ = 512          # psum node window
EPS = 1e-5
NCORES = 8
BF16 = mybir.dt.bfloat16
FP8 = mybir.dt.float8e4
F32 = mybir.dt.float32
I16 = mybir.dt.int16
S0 = 64.0          # layer-0 table scale (absorbed by BN)

USE_FP8 = os.environ.get("KGCN_FP8", "0") == "1"
HDT = FP8 if USE_FP8 else BF16
HNP = ml_dtypes.float8_e4m3fn if USE_FP8 else ml_dtypes.bfloat16


# ----------------------------------------------------------------------------
# Host preprocessing
# ----------------------------------------------------------------------------

def _wrap_idx(flat):
    n = flat.shape[0]
    assert n % 16 == 0
    w = flat.reshape(n // 16, 16).T.astype(np.int16)  # [16, n/16]
    return np.tile(w, (8, 1))


def preprocess(inputs, n_graphs=128):
    nfeat = np.asarray(inputs["nfeat"], np.int64)
    efeat = np.asarray(inputs["efeat"], np.int64)
    src = np.asarray(inputs["src"], np.int64)
    dst = np.asarray(inputs["dst"], np.int64)
    graph_ids = np.asarray(inputs["graph_ids"], np.int64)
    atom_emb = np.asarray(inputs["atom_emb"], np.float32)
    edge_emb = np.asarray(inputs["edge_emb"], np.float32)
    W = np.asarray(inputs["W"], np.float32)
    gamma = np.asarray(inputs["gamma"], np.float32)
    beta = np.asarray(inputs["beta"], np.float32)
    Wp = np.asarray(inputs["Wp"], np.float32)
    bp = np.asarray(inputs["bp"], np.float32)

    N = graph_ids.shape[0]
    E = src.shape[0]
    G = n_graphs
    GPC = G // NCORES
    AC, AV, D = atom_emb.shape
    L, BC, BV, _ = edge_emb.shape
    NCOMB = BV ** BC
    OUT = Wp.shape[1]
    HALF = NCORES // 2

    gcnt = np.bincount(graph_ids, minlength=G)
    gofs = np.concatenate([[0], np.cumsum(gcnt)])
    S = gofs[::GPC].astype(np.int64)
    assert S[-1] == N
    Nc = np.diff(S)

    NSW = int(np.ceil((Nc.max() + 1) / P))
    NPU = NSW * P
    NWIN = NPU // WSZ
    if NWIN * WSZ < NPU:
        NWIN += 1
        NPU = NWIN * WSZ
        NSW = NPU // P
    B_SPLIT = HALF * NPU
    assert B_SPLIT < 32768 and (NCORES - HALF) * NPU < 32768

    degs = np.bincount(dst, minlength=N).astype(np.float64) + 1.0
    deginv_all = (1.0 / degs).astype(np.float32)

    node_core = np.searchsorted(S[1:], np.arange(N), side="right").astype(np.int64)
    src_core = node_core[src]
    e_isL = src_core < HALF
    dLn = np.bincount(dst[e_isL], minlength=N) + (node_core < HALF)
    dHn = np.bincount(dst[~e_isL], minlength=N) + (node_core >= HALF)

    # --- per-core node permutation: balance (dL, dH) across NSW bins ---
    pos_of_node = np.full(N, -1, np.int64)
    node_at_pos = [np.full(NPU, -1, np.int64) for _ in range(NCORES)]
    for c in range(NCORES):
        nodes = np.arange(S[c], S[c + 1])
        wl = dLn[nodes].astype(np.int64)
        wh = dHn[nodes].astype(np.int64)
        order = np.argsort(-(wl + wh), kind="stable")
        binL = np.zeros(NSW, np.int64)
        binH = np.zeros(NSW, np.int64)
        binN = np.zeros(NSW, np.int64)
        for i in order:
            nl, nh = wl[i], wh[i]
            cand = np.maximum(binL + nl, binH + nh) + 1e-3 * (binL + binH)
            cand[binN >= P] = 1 << 60
            b = int(np.argmin(cand))
            slot = binN[b]
            binN[b] += 1
            binL[b] += nl
            binH[b] += nh
            n = nodes[i]
            pos_of_node[n] = c * NPU + b * P + slot
            node_at_pos[c][b * P + slot] = n

    zlo = int(np.where(node_at_pos[0] < 0)[0][0])
    zhi_core = NCORES - 1
    zhi = int(np.where(node_at_pos[zhi_core] < 0)[0][0]) + zhi_core * NPU - B_SPLIT
    assert zlo < 32768 and 0 <= zhi < 32768

    # --- edge tiling: split runs, tiles per (core, sw, stream) ---
    src_pg = pos_of_node[src]
    dst_pos = pos_of_node[dst]
    dst_core = node_core[dst]

    # per-core sorted edge streams
    core_streams = []   # [core][stream] -> (srcpos_sorted, dstlocal_sorted)
    for c in range(NCORES):
        em = dst_core == c
        es, ed, eL = src_pg[em], dst_pos[em] - c * NPU, e_isL[em]
        nodes = np.arange(S[c], S[c + 1])
        sp = pos_of_node[nodes]
        ss, sd = sp, sp - c * NPU
        sL = node_core[nodes] < HALF
        allsrc = np.concatenate([es, ss])
        alldst = np.concatenate([ed, sd])
        allL = np.concatenate([eL, sL])
        per = {}
        for stream, m in (("L", allL), ("H", ~allL)):
            ssrc = allsrc[m]
            sdst = alldst[m]
            o = np.argsort(sdst, kind="stable")
            ssrc, sdst = ssrc[o], sdst[o]
            if stream == "H":
                ssrc = ssrc - B_SPLIT
            per[stream] = (ssrc, sdst)
        core_streams.append(per)

    # tiles per (sw, stream) = global max of ceil(edges_sw / P)
    TT = {}
    for stream in ("L", "H"):
        mx = 1
        for c in range(NCORES):
            _, sdst = core_streams[c][stream]
            cnt = np.bincount(sdst // P, minlength=NSW)
            mx = max(mx, int(np.ceil(cnt.max() / P)))
        TT[stream] = mx

    def pack_core(c):
        out = {}
        for stream in ("L", "H"):
            ssrc, sdst = core_streams[c][stream]
            zrow = zlo if stream == "L" else zhi
            tt = TT[stream]
            idx = np.full((NSW, tt * P), zrow, np.int16)
            rnk = np.zeros((NSW, tt * P), np.int16)
            sw_of = sdst // P
            starts = np.concatenate([[0], np.cumsum(np.bincount(sw_of, minlength=NSW))])
            for sw in range(NSW):
                r0, r1 = starts[sw], starts[sw + 1]
                k = r1 - r0
                assert k <= tt * P
                idx[sw, :k] = ssrc[r0:r1]
                rnk[sw, :k] = sdst[r0:r1] - sw * P
            out[stream] = (idx.reshape(-1), rnk.reshape(NSW * tt, P))
        return out

    packed = [pack_core(c) for c in range(NCORES)]

    # --- tables ---
    flat_atom = np.zeros((AC * AV + 16, D), np.float32)
    flat_atom[:AC * AV] = atom_emb.reshape(AC * AV, D) * S0
    flat_atom_q = flat_atom.astype(HNP)
    ZATOM = AC * AV

    k = np.arange(NCOMB)
    d0, d1, d2 = k // (BV * BV), (k // BV) % BV, k % BV
    T512 = edge_emb[:, 0, d0] + edge_emb[:, 1, d1] + edge_emb[:, 2, d2]
    T512[0] *= S0
    T512_q = T512.astype(HNP)

    cidx = (efeat[:, 0] * BV + efeat[:, 1]) * BV + efeat[:, 2]

    cfg = dict(N=N, E=E, G=G, GPC=GPC, D=D, L=L, OUT=OUT, NPU=NPU, NSW=NSW,
               NWIN=NWIN, TT_L=TT["L"], TT_H=TT["H"],
               B_SPLIT=B_SPLIT, NCOMB=NCOMB, AC=AC, ZATOM=ZATOM, NREAL=N,
               ZLO=zlo, ZHI=zhi)

    in_maps = []
    for c in range(NCORES):
        m = {}
        for stream in ("L", "H"):
            idx, rnk = packed[c][stream]
            m[f"gidx{stream}"] = _wrap_idx(idx)
            m[f"rank{stream}"] = rnk.T.astype(ml_dtypes.bfloat16).copy()  # [P, NT]
        em = dst_core == c
        lp = dst_pos[em] - c * NPU
        ct = np.zeros((NCOMB, NPU), np.float32)
        np.add.at(ct, (cidx[em], lp), 1.0)
        m["countT"] = ct.astype(HNP)
        dg = np.zeros(NPU, np.float32)
        rp = node_at_pos[c] >= 0
        dg[rp] = deginv_all[node_at_pos[c][rp]]
        m["deginv"] = np.tile(dg[None, :], (P, 1)).astype(ml_dtypes.bfloat16)
        mk = rp.astype(np.float32).reshape(NSW, P).T.copy()
        m["maskc"] = mk.astype(ml_dtypes.bfloat16)
        sp = np.zeros((NPU, GPC), np.float32)
        gl = np.where(rp)[0]
        gid = graph_ids[node_at_pos[c][gl]] - c * GPC
        cnts = np.maximum(gcnt[c * GPC:(c + 1) * GPC], 1.0)
        sp[gl, gid] = (1.0 / cnts[gid]).astype(np.float32)
        m["selpool"] = sp
        hi = np.full((NSW, AC, P), ZATOM, np.int64)
        for st in range(NSW):
            pos = node_at_pos[c][st * P:(st + 1) * P]
            ok = pos >= 0
            nf = nfeat[pos[ok]]
            for col in range(AC):
                hi[st, col, ok] = col * AV + nf[:, col]
        m["h0idx"] = _wrap_idx(hi.reshape(-1).astype(np.int16))
        m["flat_atom"] = flat_atom_q
        m["t512"] = T512_q
        m["wl"] = W.astype(ml_dtypes.bfloat16)             # [L, D, D]
        m["gam"] = gamma.reshape(L, 1, D).copy()
        m["bet"] = beta.reshape(L, 1, D).copy()
        m["wp"] = Wp.copy()
        m["bpr"] = bp.reshape(1, OUT).copy()
        m["iota_row"] = np.tile(np.arange(P, dtype=np.float32), (P, 1)).astype(
            ml_dtypes.bfloat16)
        m["zrow"] = np.zeros((1, D), HNP)
        in_maps.append(m)

    meta = dict(S=S, Nc=Nc)
    return cfg, in_maps, meta


# ----------------------------------------------------------------------------
# Device kernel builder (uniform SPMD program)
# ----------------------------------------------------------------------------

def build(cfg):
    D = cfg["D"]; L = cfg["L"]; NPU = cfg["NPU"]; NSW = cfg["NSW"]
    NWIN = cfg["NWIN"]; TT_L = cfg["TT_L"]; TT_H = cfg["TT_H"]
    NCOMB = cfg["NCOMB"]; AC = cfg["AC"]; GPC = cfg["GPC"]; OUT = cfg["OUT"]
    B_SPLIT = cfg["B_SPLIT"]; NREAL = cfg["NREAL"]
    ZLO = cfg["ZLO"]; ZHI = cfg["ZHI"]
    KD = D // P
    SPW = WSZ // P
    NKC = NCOMB // P
    NT_L, NT_H = NSW * TT_L, NSW * TT_H
    NQ = int(os.environ.get("KGCN_NQ", "4"))

    nc = bacc.Bacc("TRN2", target_bir_lowering=False, debug=False,
                   num_devices=NCORES, num_swdge_queues=NQ)
    qrr = [0]

    def next_q():
        q = qrr[0]
        qrr[0] = (qrr[0] + 1) % NQ
        return q

    def allgather(ins, outs):
        if USE_FP8:
            ins = [ap.bitcast(BF16) for ap in ins]
            outs = [ap.bitcast(BF16) for ap in outs]
        nc.gpsimd.collective_compute(
            "AllGather", mybir.AluOpType.bypass,
            replica_groups=[list(range(NCORES))], ins=ins, outs=outs)

    def allreduce(ins, outs):
        nc.gpsimd.collective_compute(
            "AllReduce", mybir.AluOpType.add,
            replica_groups=[list(range(NCORES))], ins=ins, outs=outs)

    t_gidxL = nc.dram_tensor("gidxL", [P, NT_L * P // 16], I16, kind="ExternalInput")
    t_gidxH = nc.dram_tensor("gidxH", [P, NT_H * P // 16], I16, kind="ExternalInput")
    t_rankL = nc.dram_tensor("rankL", [P, NT_L], BF16, kind="ExternalInput")
    t_rankH = nc.dram_tensor("rankH", [P, NT_H], BF16, kind="ExternalInput")
    t_countT = nc.dram_tensor("countT", [NCOMB, NPU], HDT, kind="ExternalInput")
    t_deginv = nc.dram_tensor("deginv", [P, NPU], BF16, kind="ExternalInput")
    t_mask = nc.dram_tensor("maskc", [P, NSW], BF16, kind="ExternalInput")
    t_selpool = nc.dram_tensor("selpool", [NPU, GPC], F32, kind="ExternalInput")
    t_h0idx = nc.dram_tensor("h0idx", [P, NSW * AC * P // 16], I16, kind="ExternalInput")
    t_atom = nc.dram_tensor("flat_atom", [AC * 128 + 16, D], HDT, kind="ExternalInput")
    t_t512 = nc.dram_tensor("t512", [L, NCOMB, D], HDT, kind="ExternalInput")
    t_wl = nc.dram_tensor("wl", [L, D, D], BF16, kind="ExternalInput")
    t_gam = nc.dram_tensor("gam", [L, 1, D], F32, kind="ExternalInput")
    t_bet = nc.dram_tensor("bet", [L, 1, D], F32, kind="ExternalInput")
    t_wp = nc.dram_tensor("wp", [D, OUT], F32, kind="ExternalInput")
    t_bp = nc.dram_tensor("bpr", [1, OUT], F32, kind="ExternalInput")
    t_iota = nc.dram_tensor("iota_row", [P, P], BF16, kind="ExternalInput")
    t_zrow = nc.dram_tensor("zrow", [1, D], HDT, kind="ExternalInput")
    t_out = nc.dram_tensor("out_g", [GPC, OUT], F32, kind="ExternalOutput")
    shared = os.environ.get("KGCN_SHARED", "0") == "1"
    t_hfull = nc.dram_tensor("h_full", [NCORES * NPU, D], HDT,
                             **({"addr_space": "Shared"} if shared else {}))
    t_hnew = nc.dram_tensor("h_newc", [NPU, D], HDT)
    DBG = os.environ.get("KGCN_DBG", "0") == "1"
    t_arin = [nc.dram_tensor(f"arin{l}", [2, D], F32) for l in range(L)]
    t_arout = [nc.dram_tensor(f"arout{l}", [2, D], F32) for l in range(L)]
    t_argo = [nc.dram_tensor(f"argout{l}", [2, D], F32, kind="ExternalOutput")
              for l in range(L)] if DBG else None
    t_dbg = (nc.dram_tensor("dbg_h", [L, NPU, D], HDT, kind="ExternalOutput")
             if DBG else None)

    with tile.TileContext(nc) as tc:
        with (
            tc.tile_pool(name="static", bufs=1) as stp,
            tc.tile_pool(name="gath", bufs=3) as gpool,
            tc.tile_pool(name="selp", bufs=3) as selpool_p,
            tc.tile_pool(name="xt", bufs=4) as xtp,
            tc.tile_pool(name="work", bufs=3) as wk,
            tc.tile_pool(name="small", bufs=1) as smp,
            tc.tile_pool(name="winps", bufs=2, space="PSUM") as wps,
            tc.tile_pool(name="hlps", bufs=1, space="PSUM") as hps,
            tc.tile_pool(name="smps", bufs=1, space="PSUM") as sps,
            tc.tile_pool(name="abps", bufs=1, space="PSUM") as aps,
        ):
            # ---- static SBUF preloads ----
            gidxL = stp.tile([P, NT_L * P // 16], I16)
            gidxH = stp.tile([P, NT_H * P // 16], I16)
            rankL = stp.tile([P, NT_L], BF16)
            rankH = stp.tile([P, NT_H], BF16)
            h0idx = stp.tile([P, NSW * AC * P // 16], I16)
            maskS = stp.tile([P, NSW], BF16)
            selpS = stp.tile([P, NSW, GPC], F32)
            dgS = stp.tile([P, NPU], BF16)
            wS = stp.tile([P, L, KD, D], BF16)
            t5S = stp.tile([P, L, NKC, D], HDT)
            gamS = stp.tile([1, L, D], F32)
            betS = stp.tile([1, L, D], F32)
            wpS = stp.tile([P, KD, OUT], F32)
            bpS = stp.tile([1, OUT], F32)
            onesS = stp.tile([1, P], F32)
            iotaS = stp.tile([P, P], BF16)
            zrowS = stp.tile([1, D], HDT)
            hlinS = stp.tile([P, NSW, D], BF16)
            epsS = stp.tile([1, 1], F32)
            nc.vector.memset(epsS[:], EPS)
            nc.sync.dma_start(iotaS[:], t_iota[:])
            nc.sync.dma_start(gidxL[:], t_gidxL[:])
            nc.sync.dma_start(gidxH[:], t_gidxH[:])
            nc.sync.dma_start(rankL[:], t_rankL[:])
            nc.sync.dma_start(rankH[:], t_rankH[:])
            nc.sync.dma_start(h0idx[:], t_h0idx[:])
            nc.sync.dma_start(maskS[:], t_mask[:])
            nc.sync.dma_start(selpS[:], t_selpool.ap().rearrange("(s p) g -> p s g", p=P))
            nc.sync.dma_start(dgS[:], t_deginv[:])
            nc.sync.dma_start(wS[:], t_wl.ap().rearrange("l (k p) d -> p l k d", p=P))
            nc.sync.dma_start(t5S[:], t_t512.ap().rearrange("l (k p) d -> p l k d", p=P))
            nc.sync.dma_start(gamS[:], t_gam.ap().rearrange("l o d -> o l d"))
            nc.sync.dma_start(betS[:], t_bet.ap().rearrange("l o d -> o l d"))
            nc.sync.dma_start(wpS[:], t_wp.ap().rearrange("(k p) o -> p k o", p=P))
            nc.sync.dma_start(bpS[:], t_bp[:])
            nc.sync.dma_start(zrowS[:], t_zrow[:])
            nc.vector.memset(onesS[:], 1.0)

            # ================= h0: atom embedding sums =================
            for st in range(NSW):
                nidx = AC * P
                g = gpool.tile([P, AC, D], HDT, tag="g0", bufs=2)
                nc.gpsimd.dma_gather(
                    g[:], t_atom[:],
                    h0idx[:, st * (nidx // 16):(st + 1) * (nidx // 16)],
                    nidx, nidx, D, single_packet=False, queue_num=next_q())
                acc = wk.tile([P, 4, D], BF16, tag="h0acc", bufs=2)
                nc.vector.tensor_tensor(out=acc[:], in0=g[:, 0:4, :],
                                        in1=g[:, 4:8, :], op=mybir.AluOpType.add)
                acc2 = wk.tile([P, 2, D], BF16, tag="h0acc2", bufs=2)
                nc.vector.tensor_tensor(out=acc2[:], in0=acc[:, 0:2, :],
                                        in1=acc[:, 2:4, :], op=mybir.AluOpType.add)
                h0t = wk.tile([P, D], BF16, tag="h0t", bufs=2)
                nc.vector.tensor_tensor(out=h0t[:], in0=acc2[:, 0, :],
                                        in1=acc2[:, 1, :], op=mybir.AluOpType.add)
                h0q = wk.tile([P, D], HDT, tag="h0q", bufs=2)
                nc.vector.tensor_tensor(out=h0q[:], in0=h0t[:],
                                        in1=g[:, 8, :], op=mybir.AluOpType.add)
                nc.sync.dma_start(t_hnew[st * P:(st + 1) * P, :], h0q[:])
            if DBG:
                nc.sync.dma_start(t_dbg[0], t_hnew[:])
            allgather([t_hnew[:]], [t_hfull[:]])

            # ================= layers =================
            for l in range(L):
                stats0 = sps.tile([1, D], F32, tag="stats0")
                stats1 = sps.tile([1, D], F32, tag="stats1")
                if l == L - 1:
                    poolps = [sps.tile([P, GPC], F32, tag=f"pool{h}",
                                       name=f"pool{h}") for h in range(KD)]
                for w in range(NWIN):
                    winp = [wps.tile([P, WSZ], F32, tag="win", name=f"win{h}")
                            for h in range(KD)]
                    # bond-encoder first: no dependence on h_full
                    ctk = wk.tile([P, NKC, WSZ], HDT, tag="ct", bufs=2)
                    nc.sync.dma_start(
                        ctk[:], t_countT.ap().rearrange(
                            "(k p) n -> p k n", p=P)[:, :, w * WSZ:(w + 1) * WSZ])
                    for sw in range(SPW):
                        gsw = w * SPW + sw
                        # bond term first (contiguous with this sw's group:
                        # start flags the whole 2KB zero-region, so a region's
                        # accumulation group must not interleave with another
                        # region's start)
                        for kk in range(NKC):
                            for h in range(KD):
                                nc.tensor.matmul(
                                    out=winp[h][:, sw * P:(sw + 1) * P],
                                    lhsT=t5S[:, l, kk, h * P:(h + 1) * P],
                                    rhs=ctk[:, kk, sw * P:(sw + 1) * P],
                                    start=(kk == 0), stop=False)
                        for stream, tt, gidx, rank in (
                                ("L", TT_L, gidxL, rankL),
                                ("H", TT_H, gidxH, rankH)):
                            nidx = tt * P
                            gt = gpool.tile([P, tt, D], HDT,
                                            tag=f"g{stream}",
                                            name=f"g{stream}t", bufs=4)
                            tbl = (t_hfull[0:B_SPLIT, :] if stream == "L"
                                   else t_hfull[B_SPLIT:NCORES * NPU, :])
                            nc.gpsimd.dma_gather(
                                gt[:], tbl,
                                gidx[:, gsw * (nidx // 16):(gsw + 1) * (nidx // 16)],
                                nidx, nidx, D, single_packet=False,
                                queue_num=next_q())
                            sel = selpool_p.tile([P, tt, P], HDT,
                                                 tag=f"s{stream}",
                                                 name=f"s{stream}t")
                            rk = rank[:, gsw * tt:(gsw + 1) * tt]
                            in0 = bass.AP(rk.tensor, rk.offset,
                                          [rk.ap[0], list(rk.ap[1]), [0, P]])
                            io = iotaS[:]
                            in1 = bass.AP(io.tensor, io.offset,
                                          [io.ap[0], [0, tt], [1, P]])
                            nc.vector.tensor_tensor(
                                out=sel[:], in0=in0, in1=in1,
                                op=mybir.AluOpType.is_equal)
                            last_stream = stream == "H"
                            for t in range(tt):
                                for h in range(KD):
                                    nc.tensor.matmul(
                                        out=winp[h][:, sw * P:(sw + 1) * P],
                                        lhsT=gt[:, t, h * P:(h + 1) * P],
                                        rhs=sel[:, t, :],
                                        start=False,
                                        stop=(last_stream and t == tt - 1))
                    # x^T = deginv * window  (bf16)
                    xt = [xtp.tile([P, WSZ], BF16, tag="xt", name=f"xt{h}")
                          for h in range(KD)]
                    for h in range(KD):
                        nc.vector.tensor_tensor(
                            out=xt[h][:], in0=winp[h][:],
                            in1=dgS[:, w * WSZ:(w + 1) * WSZ],
                            op=mybir.AluOpType.mult)
                    # update matmul + stats per subtile
                    for sw in range(SPW):
                        st = w * SPW + sw
                        hlp = hps.tile([P, D], F32, tag="hl")
                        for h in range(KD):
                            nc.tensor.matmul(
                                out=hlp[:],
                                lhsT=xt[h][:, sw * P:(sw + 1) * P],
                                rhs=wS[:, l, h, :],
                                start=(h == 0), stop=(h == KD - 1))
                        nc.scalar.activation(hlinS[:, st, :], hlp[:],
                                             mybir.ActivationFunctionType.Copy)
                        sq = wk.tile([P, D], BF16, tag="sq")
                        nc.vector.tensor_tensor(out=sq[:], in0=hlinS[:, st, :],
                                                in1=hlinS[:, st, :],
                                                op=mybir.AluOpType.mult)
                        nc.tensor.matmul(out=stats0[:],
                                         lhsT=maskS[:, st:st + 1],
                                         rhs=hlinS[:, st, :],
                                         start=(st == 0), stop=(st == NSW - 1))
                        nc.tensor.matmul(out=stats1[:],
                                         lhsT=maskS[:, st:st + 1], rhs=sq[:],
                                         start=(st == 0), stop=(st == NSW - 1))
                # --- BN stats allreduce + scale/shift ---
                stsb0 = smp.tile([1, D], F32, tag="stsb0")
                stsb1 = smp.tile([1, D], F32, tag="stsb1")
                nc.scalar.activation(stsb0[:], stats0[:],
                                     mybir.ActivationFunctionType.Copy)
                nc.scalar.activation(stsb1[:], stats1[:],
                                     mybir.ActivationFunctionType.Copy)
                nc.sync.dma_start(t_arin[l][0:1, :], stsb0[:])
                nc.sync.dma_start(t_arin[l][1:2, :], stsb1[:])
                allreduce([t_arin[l][:]], [t_arout[l][:]])
                stg0 = smp.tile([1, D], F32, tag="stg0")
                stg1 = smp.tile([1, D], F32, tag="stg1")
                nc.sync.dma_start(stg0[:], t_arout[l][0:1, :])
                nc.sync.dma_start(stg1[:], t_arout[l][1:2, :])
                if DBG:
                    nc.sync.dma_start(t_argo[l][0:1, :], stg0[:])
                    nc.sync.dma_start(t_argo[l][1:2, :], stg1[:])
                mean = smp.tile([1, D], F32, tag="mean")
                nc.vector.tensor_scalar_mul(mean[:], stg0[:], 1.0 / NREAL)
                msq = smp.tile([1, D], F32, tag="msq")
                nc.vector.tensor_scalar_mul(msq[:], stg1[:], 1.0 / NREAL)
                var = smp.tile([1, D], F32, tag="var")
                nc.vector.tensor_tensor(out=var[:], in0=mean[:], in1=mean[:],
                                        op=mybir.AluOpType.mult)
                nc.vector.tensor_tensor(out=var[:], in0=msq[:], in1=var[:],
                                        op=mybir.AluOpType.subtract)
                sd = smp.tile([1, D], F32, tag="sd")
                nc.scalar.activation(sd[:], var[:],
                                     mybir.ActivationFunctionType.Sqrt,
                                     bias=epsS[:])
                rsq = smp.tile([1, D], F32, tag="rsq")
                nc.vector.reciprocal(rsq[:], sd[:])
                scl = smp.tile([1, D], F32, tag="scl")
                nc.vector.tensor_tensor(out=scl[:], in0=rsq[:],
                                        in1=gamS[:, l, :],
                                        op=mybir.AluOpType.mult)
                sft = smp.tile([1, D], F32, tag="sft")
                nc.vector.tensor_tensor(out=sft[:], in0=mean[:], in1=scl[:],
                                        op=mybir.AluOpType.mult)
                nc.vector.tensor_tensor(out=sft[:], in0=betS[:, l, :],
                                        in1=sft[:],
                                        op=mybir.AluOpType.subtract)
                ab = aps.tile([P, 2 * D], F32, tag="ab")
                nc.tensor.matmul(out=ab[:, 0:D], lhsT=onesS[:], rhs=scl[:],
                                 start=True, stop=True)
                nc.tensor.matmul(out=ab[:, D:2 * D], lhsT=onesS[:], rhs=sft[:],
                                 start=True, stop=True)
                # --- apply + (layer L-1) pooling ---
                for st in range(NSW):
                    hnf = wk.tile([P, D], F32, tag="hnf")
                    nc.vector.tensor_tensor(out=hnf[:], in0=hlinS[:, st, :],
                                            in1=ab[:, 0:D],
                                            op=mybir.AluOpType.mult)
                    nc.vector.tensor_tensor(out=hnf[:], in0=hnf[:],
                                            in1=ab[:, D:2 * D],
                                            op=mybir.AluOpType.add)
                    if l < L - 1:
                        hnb = wk.tile([P, D], HDT, tag="hnb")
                        nc.vector.tensor_scalar_max(hnb[:], hnf[:], 0.0)
                        nc.sync.dma_start(t_hnew[st * P:(st + 1) * P, :], hnb[:])
                    else:
                        hnr = wk.tile([P, D], F32, tag="hnr")
                        nc.vector.tensor_scalar_max(hnr[:], hnf[:], 0.0)
                        nc.vector.tensor_tensor(
                            out=hnr[:], in0=hnr[:],
                            in1=maskS[:, st:st + 1].to_broadcast([P, D]),
                            op=mybir.AluOpType.mult)
                        for h in range(KD):
                            nc.tensor.matmul(
                                out=poolps[h][:],
                                lhsT=hnr[:, h * P:(h + 1) * P],
                                rhs=selpS[:, st, :],
                                start=(st == 0), stop=(st == NSW - 1))
                if DBG and l < L - 1:
                    nc.sync.dma_start(t_dbg[l + 1], t_hnew[:])
                if l < L - 1:
                    allgather([t_hnew[:]], [t_hfull[:]])
                    # re-zero the dead-row targets (pad rows get BN shift)
                    nc.sync.dma_start(t_hfull[ZLO:ZLO + 1, :], zrowS[:])
                    nc.sync.dma_start(
                        t_hfull[B_SPLIT + ZHI:B_SPLIT + ZHI + 1, :], zrowS[:])

            # ================= readout =================
            gts = smp.tile([P, KD * GPC], F32, tag="gts")
            for h in range(KD):
                nc.scalar.activation(gts[:, h * GPC:(h + 1) * GPC],
                                     poolps[h][:],
                                     mybir.ActivationFunctionType.Copy)
            ones16 = smp.tile([1, GPC], F32, tag="o16")
            nc.vector.memset(ones16[:], 1.0)
            outp = sps.tile([GPC, OUT], F32, tag="stats0")
            for h in range(KD):
                nc.tensor.matmul(out=outp[:],
                                 lhsT=gts[:, h * GPC:(h + 1) * GPC],
                                 rhs=wpS[:, h, :], start=(h == 0), stop=False)
            nc.tensor.matmul(out=outp[:], lhsT=ones16[:], rhs=bpS[:],
                             start=False, stop=True)
            outs = smp.tile([GPC, OUT], F32, tag="outs")
            nc.scalar.activation(outs[:], outp[:],
                                 mybir.ActivationFunctionType.Copy)
            nc.sync.dma_start(t_out[:], outs[:])

    nc.compile()
    return nc


LAST = {}


def kernel(**inputs):
    cfg, in_maps, _ = preprocess(inputs)
    nc = build(cfg)
    trace = os.environ.get("KGCN_TRACE") == "1"
    res = run_bass_kernel_spmd(nc, in_maps, list(range(NCORES)), trace=trace)
    LAST["exec_time_ns"] = res.exec_time_ns
    LAST["profile_json"] = res.profile_json
    out = np.concatenate([res.results[c]["out_g"] for c in range(NCORES)], 0)
    return out.astype(np.float32)


if __name__ == "__main__":
    pass

